# revision 38
# baseline (speedup 1.0000x reference)
"""GCNConv on 8 Trainium2 NeuronCores (Bass/Tile) — transfer-pipelined.

The device compute (projection + edge aggregation) takes <100ms; the axon
tunnel (~40MB/s each way) dominates, so the kernel is engineered around the
wire:

 - x is quantized host-side to int8 with per-row scales (12.8MB up instead
   of 25.7MB bf16); the dequant scale is fused into the projection's
   PSUM->SBUF copy (activation scale).
 - The output is quantized on-device to int8 with per-row scales packed as
   4 extra bytes per row (13.2MB down instead of 25.7MB), dequantized on
   the host. Measured end-to-end rel err 0.9e-2 vs the 2e-2 gate.
 - No donation: output params are persistent device-resident zero buffers,
   so no 25.7MB zero upload per call.
 - Work is split into four stage invocations of one program (each
   re-projects + AllGathers h, then aggregates a quarter of the
   destination tiles) so early stages' output fetches overlap later
   stages' uploads/exec on the full-duplex tunnel.
 - Edge values are quantized to int8 with one global scale (uniform [0,1)
   values suit linear fixed-point; fp8's log spacing fails the gate). The
   x-row scales, W (bf16), and the vals scale are bit-packed as extra
   int8 columns of the xq upload and read via SBUF AP bitcasts, so each
   core's entire dense payload is one device_put.
 - Per-(core,stage) edge payloads are packed into single int8 blobs
   (gidx int16 region | dest-local int8 region | vals int8 region, read on
   device through SBUF AP bitcasts) and uploaded with async device_puts
   issued core-by-core the moment the host finishes each piece, so the
   wire never idles behind host prep.

Aggregation math is the baseline's exact scheme: edges bucketed by
(dest-core, stage-tile, col-chunk) with fixed CAP=640 per bucket (seed-0
max occupancy is 595; overflow is corrected exactly on the host), gpsimd
dma_gather pulls h[cols] per 640-edge bucket, the vector engine scales by
vals and builds selection matrices, and the tensor engine accumulates
S^T @ M into one PSUM tile per dest tile — an exact f32 segment-sum.
"""
import ctypes
import sys

import numpy as np
import ml_dtypes

try:
    # keep large numpy buffers on the heap arena across calls: without this
    # glibc mmap/munmaps every >128KB allocation, so each kernel() call
    # page-faults ~80MB of fresh temporaries inside the timed region
    _libc = ctypes.CDLL("libc.so.6", use_errno=True)
    _libc.mallopt(-3, 1 << 28)   # M_MMAP_THRESHOLD = 256MB
    _libc.mallopt(-1, 1 << 28)   # M_TRIM_THRESHOLD = 256MB
except Exception:
    pass

sys.path.insert(0, "/opt/trn_rl_repo")

import concourse.bass as bass
import concourse.bacc as bacc
import concourse.mybir as mybir
import concourse.tile as tile

F32 = mybir.dt.float32
BF16 = mybir.dt.bfloat16
I16 = mybir.dt.int16
I8 = mybir.dt.int8

N_NODES = 100000
D = 128
NCORES = 8
SHARD = 12544                  # 98 tiles of 128 rows per core
N_TILES = SHARD // D
N_PAD = SHARD * NCORES         # 100352
CHUNK = 25088                  # col-index window (int16-safe)
N_CHUNKS = N_PAD // CHUNK
CAP = 640                      # edges per (dest tile, col chunk) bucket
NSTAGES = 4
T_STAGE = 25                   # tiles per stage (98 real + 2 pad)
NB_S = T_STAGE * N_CHUNKS      # buckets per (core, stage), tile-major
ETOT_S = NB_S * CAP            # padded edge slots per (core, stage)
EB = 4 * ETOT_S                # blob bytes: gidx 2E | dl 1E | vals 1E
WSC_COLS = N_TILES + 64 + 1    # xsc f32 | W.T bf16 bitcast | vals scale
OCOLS = D + 4                  # int8 data + f32 scale bytes per row
OROWS = T_STAGE * D            # 3200 output rows per (core, stage)
XQW_COLS = SHARD + 4 * WSC_COLS  # xq int8 | wsc f32 bit-packed per core
MAGIC = 12582912.0             # 1.5 * 2**23: fast float32 round-to-int


def _build_nc():
    """Stage program: project h = x @ W.T (dequantizing int8 x), AllGather
    h, aggregate T_STAGE dest tiles from the edge blob (tile-major, one
    640-edge gather per (tile, chunk) bucket, one PSUM tile per dest
    tile), emit int8 output rows with packed per-row f32 scales."""
    nc = bacc.Bacc("TRN2", target_bir_lowering=False, debug=False,
                   num_devices=NCORES)

    xq = nc.dram_tensor("xq", [D, XQW_COLS], I8, kind="ExternalInput")
    eblob = nc.dram_tensor("eblob", [128, EB // 128], I8,
                           kind="ExternalInput")
    iota = nc.dram_tensor("iota", [128, 128], I8, kind="ExternalInput")
    outq = nc.dram_tensor("outq", [OROWS, OCOLS], I8, kind="ExternalOutput")

    h_local = nc.dram_tensor("h_local", [SHARD, D], BF16)
    h_full = nc.dram_tensor("h_full", [N_PAD, D], BF16, addr_space="Shared")

    NBLK = CAP // 128          # 5 gather blocks per bucket

    with tile.TileContext(nc) as tc:
        with (
            tc.tile_pool(name="big", bufs=1) as big_pool,
            tc.tile_pool(name="proj", bufs=4) as proj_pool,
            tc.tile_pool(name="idx", bufs=3) as idx_pool,
            tc.tile_pool(name="gat", bufs=2) as gat_pool,
            tc.tile_pool(name="sca", bufs=2) as sca_pool,
            tc.tile_pool(name="sel", bufs=2) as sel_pool,
            tc.tile_pool(name="psum", bufs=4,
                         space=bass.MemorySpace.PSUM) as psum_pool,
            tc.tile_pool(name="fin", bufs=4) as fin_pool,
            tc.tile_pool(name="qs", bufs=4) as qs_pool,
        ):
            xq_sb = big_pool.tile([D, XQW_COLS], I8)
            dl_sb = big_pool.tile([128, ETOT_S // 128], I8)
            va_sb = big_pool.tile([128, ETOT_S // 128], I8)
            vab = big_pool.tile([128, ETOT_S // 128], BF16)
            iota_sb = big_pool.tile([128, 128], I8)

            nc.sync.dma_start(xq_sb[:], xq[:])
            nc.sync.dma_start(
                dl_sb[:],
                bass.AP(eblob, 2 * ETOT_S,
                        [[ETOT_S // 128, 128], [1, ETOT_S // 128]]),
            )
            nc.sync.dma_start(
                va_sb[:],
                bass.AP(eblob, 3 * ETOT_S,
                        [[ETOT_S // 128, 128], [1, ETOT_S // 128]]),
            )
            nc.sync.dma_start(iota_sb[:], iota[:])
            # bitcast views into the wsc region packed after the xq columns
            wt_bf = xq_sb[:, SHARD + 4 * N_TILES:
                          SHARD + 4 * (N_TILES + 64)].bitcast(BF16)

            def _xsc(t):
                return xq_sb[:, SHARD + 4 * t:SHARD + 4 * (t + 1)].bitcast(F32)

            # dequantize int8 vals with the global scale (wsc last col)
            nc.scalar.activation(
                vab[:], va_sb[:], mybir.ActivationFunctionType.Copy,
                scale=_xsc(WSC_COLS - 1),
            )

            # projection: h_local = (xq @ W.T) * xscale, 128-row tiles
            for t in range(N_TILES):
                xb = proj_pool.tile([D, D], BF16)
                nc.scalar.copy(xb[:], xq_sb[:, t * D:(t + 1) * D])
                ps = psum_pool.tile([D, D], F32)
                nc.tensor.matmul(out=ps[:], lhsT=xb[:], rhs=wt_bf,
                                 start=True, stop=True)
                ht = proj_pool.tile([D, D], BF16)
                nc.scalar.activation(
                    ht[:], ps[:], mybir.ActivationFunctionType.Copy,
                    scale=_xsc(t),
                )
                nc.sync.dma_start(
                    bass.AP(h_local, t * D * D, [[D, D], [1, D]]), ht[:]
                )

            nc.gpsimd.collective_compute(
                "AllGather",
                mybir.AluOpType.bypass,
                replica_groups=[list(range(NCORES))],
                ins=[h_local[:]],
                outs=[h_full[:]],
            )

            # aggregation: tile-major, one 640-edge bucket per (tile, chunk)
            for t in range(T_STAGE):
                ps = psum_pool.tile([D, D], F32)
                for k in range(N_CHUNKS):
                    stream_off = (t * N_CHUNKS + k) * CAP
                    v0 = stream_off // 128
                    gq = idx_pool.tile([128, CAP // 8], I8)
                    nc.sync.dma_start(
                        gq[:],
                        bass.AP(eblob, stream_off // 8,
                                [[0, 8], [ETOT_S // 8, 16], [1, CAP // 8]]),
                    )
                    g = gat_pool.tile([128, NBLK, D], BF16)
                    nc.gpsimd.dma_gather(
                        out_ap=g[:],
                        in_ap=bass.AP(h_full, k * CHUNK * D,
                                      [[D, CHUNK], [1, D]]),
                        idxs_ap=gq[:].bitcast(I16),
                        num_idxs=CAP,
                        num_idxs_reg=CAP,
                        elem_size=D,
                    )
                    gs = sca_pool.tile([128, NBLK, D], BF16)
                    nc.vector.tensor_tensor(
                        out=gs[:],
                        in0=g[:],
                        in1=vab[:, v0:v0 + NBLK]
                        .unsqueeze(2).to_broadcast([128, NBLK, D]),
                        op=mybir.AluOpType.mult,
                    )
                    sel = sel_pool.tile([128, NBLK, D], BF16)
                    nc.vector.tensor_tensor(
                        out=sel[:],
                        in0=dl_sb[:, v0:v0 + NBLK]
                        .unsqueeze(2).to_broadcast([128, NBLK, D]),
                        in1=iota_sb[:].unsqueeze(1)
                        .to_broadcast([128, NBLK, D]),
                        op=mybir.AluOpType.is_equal,
                    )
                    for j in range(NBLK):
                        nc.tensor.matmul(
                            out=ps[:],
                            lhsT=sel[:, j, :],
                            rhs=gs[:, j, :],
                            start=(k == 0 and j == 0),
                            stop=(k == N_CHUNKS - 1 and j == NBLK - 1),
                        )

                # int8 quantization with per-row scale, packed [q | scale]
                m = qs_pool.tile([128, 1], F32)
                nc.vector.tensor_reduce(
                    out=m[:], in_=ps[:],
                    axis=mybir.AxisListType.X, op=mybir.AluOpType.max,
                    apply_absolute_value=True,
                )
                nc.vector.tensor_scalar_max(m[:], m[:], 1e-30)
                inv = qs_pool.tile([128, 1], F32)
                nc.vector.reciprocal(inv[:], m[:])
                inv127 = qs_pool.tile([128, 1], F32)
                nc.vector.tensor_scalar_mul(inv127[:], inv[:], 127.0)
                fin = fin_pool.tile([128, OCOLS], I8)
                nc.vector.tensor_scalar(
                    out=fin[:, 0:D], in0=ps[:],
                    scalar1=inv127[:], scalar2=None,
                    op0=mybir.AluOpType.mult,
                )
                nc.scalar.mul(fin[:, D:OCOLS].bitcast(F32), m[:], 1.0 / 127.0)
                nc.sync.dma_start(
                    bass.AP(outq, t * D * OCOLS, [[OCOLS, D], [1, OCOLS]]),
                    fin[:],
                )

    nc.compile()
    return nc


_NC = _build_nc()


def _make_runner(nc):
    """Jitted no-donation executor for the stage program."""
    import jax
    from jax.sharding import Mesh, PartitionSpec
    from jax.experimental.shard_map import shard_map
    from concourse import bass2jax

    bass2jax.install_neuronx_cc_hook()
    assert nc.dbg_addr is None

    partition_name = (nc.partition_id_tensor.name
                      if nc.partition_id_tensor else None)
    in_names, out_names, out_avals = [], [], []
    for alloc in nc.m.functions[0].allocations:
        if not isinstance(alloc, mybir.MemoryLocationSet):
            continue
        name = alloc.memorylocations[0].name
        if alloc.kind == "ExternalInput":
            if name != partition_name:
                in_names.append(name)
        elif alloc.kind == "ExternalOutput":
            shape = tuple(alloc.tensor_shape)
            dtype = mybir.dt.np(alloc.dtype)
            out_names.append(name)
            out_avals.append(jax.core.ShapedArray(shape, dtype))
    n_params = len(in_names)
    all_in = in_names + out_names
    if partition_name is not None:
        all_in.append(partition_name)

    def _body(*args):
        operands = list(args)
        if partition_name is not None:
            operands.append(bass2jax.partition_id_tensor())
        outs = bass2jax._bass_exec_p.bind(
            *operands,
            out_avals=tuple(out_avals),
            in_names=tuple(all_in),
            out_names=tuple(out_names),
            lowering_input_output_aliases=(),
            sim_require_finite=True,
            sim_require_nnan=True,
            nc=nc,
        )
        return tuple(outs)

    devices = jax.devices()[:NCORES]
    mesh = Mesh(np.asarray(devices), ("core",))
    spec = PartitionSpec("core")
    sharded = jax.jit(
        shard_map(_body, mesh=mesh, in_specs=(spec,) * (n_params + 1),
                  out_specs=(spec,), check_rep=False),
        keep_unused=True,
    )
    return sharded


_IOTA8 = np.ascontiguousarray(
    np.broadcast_to(np.arange(128, dtype=np.int8), (128, 128)))

# per-row-tile key LUT: rows >> 7 -> core*NSTAGES*NB_S + stage*NB_S + tls*4
_R7 = np.arange(N_PAD // D, dtype=np.int32)
_R7_TL = _R7 % N_TILES
_KEYROW = (_R7 // N_TILES * (NSTAGES * NB_S)
           + (_R7_TL // T_STAGE) * NB_S
           + (_R7_TL % T_STAGE) * N_CHUNKS).astype(np.int16)


class _Device:
    """Holds the jit and the persistent device-resident constants."""

    def __init__(self):
        import jax
        from jax.sharding import Mesh, NamedSharding, PartitionSpec

        self.jax = jax
        self.devices = jax.devices()[:NCORES]
        mesh = Mesh(np.asarray(self.devices), ("core",))
        self.sh = NamedSharding(mesh, PartitionSpec("core"))
        self.runner = _make_runner(_NC)
        self.iota_dev = jax.device_put(
            np.concatenate([_IOTA8] * NCORES, axis=0), self.sh)
        self.zout_dev = jax.device_put(
            np.zeros((NCORES * OROWS, OCOLS), np.int8), self.sh)
        jax.block_until_ready(self.iota_dev)
        jax.block_until_ready(self.zout_dev)

    def put_global(self, arr):
        return self.jax.device_put(arr, self.sh)


try:
    _DEV = _Device()
except Exception:
    _DEV = None


def _warmup():
    if _DEV is None:
        return
    try:
        # random payloads so the transfer path warms at true (incompressible)
        # sizes — zero-filled warmups compress on the wire and leave the
        # first real call paying relay buffer growth
        rng = np.random.default_rng(0)
        e = np.arange(1600000, dtype=np.int32)
        synth = {
            "x": rng.standard_normal((N_NODES, D)).astype(np.float32),
            "W": rng.standard_normal((D, D)).astype(np.float32) * 0.09,
            "adj_rows": e % N_NODES,
            "adj_cols": (e * 7) % N_NODES,
            "adj_vals": rng.random(1600000, dtype=np.float32),
        }
        for _ in range(2):
            kernel(**synth)
    except Exception:
        pass


def _host_reference(x, W, rows, cols, vals):
    """Exact full-host fallback (used only if the device path fails)."""
    h = x @ W.T
    order = np.argsort(rows, kind="stable")
    rows_s = rows[order]
    msg = h[cols[order]] * vals[order][:, None]
    boundaries = np.searchsorted(rows_s, np.arange(N_NODES)).astype(np.int64)
    np.clip(boundaries, 0, max(len(rows_s) - 1, 0), out=boundaries)
    out = np.add.reduceat(msg, boundaries, axis=0)
    counts = np.bincount(rows, minlength=N_NODES)
    out[counts == 0] = 0.0
    return out.astype(np.float32)


def kernel(x, W, adj_rows, adj_cols, adj_vals):
    x = np.asarray(x, dtype=np.float32)
    W = np.asarray(W, dtype=np.float32)
    rows = np.asarray(adj_rows).astype(np.int32, copy=False)
    cols = np.asarray(adj_cols).astype(np.int32, copy=False)
    vals = np.asarray(adj_vals, dtype=np.float32)
    n = x.shape[0]

    if _DEV is None:
        return _host_reference(x, W, rows, cols, vals)

    jax = _DEV.jax

    # ---- x: per-row int8 quantization + packed scales/W, per-core
    # async upload pipeline (wsc region bit-packed after the xq columns)
    vsc = max(float(np.abs(vals).max()), 1e-30) / 127.0
    wtb = np.ascontiguousarray(W.T).astype(ml_dtypes.bfloat16)
    wtf = wtb.view(np.float32)               # [128, 64] bit-packed bf16
    wsc_c = np.empty((128, WSC_COLS), np.float32)
    wsc_c[:, N_TILES:N_TILES + 64] = wtf
    wsc_c[:, WSC_COLS - 1] = vsc
    xq_parts = []
    for c in range(NCORES):
        lo = c * SHARD
        hi = min(lo + SHARD, n)
        xc = x[lo:hi]
        m = np.abs(xc).max(axis=1)
        np.maximum(m, 1e-30, out=m)
        s = m * (1.0 / 127.0)
        buf = xc * (1.0 / s)[:, None]
        buf += MAGIC
        i32 = buf.view(np.int32)
        i32 -= 0x4B400000
        q8 = i32.astype(np.int8)
        qt = np.zeros((D, XQW_COLS), np.int8)
        qt[:, :xc.shape[0]] = q8.T
        s_pad = np.full(SHARD, 1.0, np.float32)
        s_pad[:xc.shape[0]] = s
        wsc_c[:, :N_TILES] = s_pad.reshape(N_TILES, 128).T
        qt[:, SHARD:] = wsc_c.view(np.int8)
        xq_parts.append(jax.device_put(qt, _DEV.devices[c]))
    xq_dev = jax.make_array_from_single_device_arrays(
        (NCORES * D, XQW_COLS), _DEV.sh, xq_parts)

    # ---- edge bucketing: (core, stage, tile, chunk) with CAP slots each
    E = rows.shape[0]
    key = _KEYROW[rows >> 7] + (cols // CHUNK).astype(np.int16)
    order = np.argsort(key, kind="stable")
    key_s = key[order]
    dl_s = (rows & 127).astype(np.int8)[order]
    cw_s = (cols % CHUNK).astype(np.int16)[order]
    vq = vals * (1.0 / vsc)
    vq += MAGIC
    vqi = vq.view(np.int32)
    vqi -= 0x4B400000
    va_s = vqi.astype(np.int8)[order]

    nkeys = NCORES * NSTAGES * NB_S
    cnt = np.bincount(key_s, minlength=nkeys)
    startb = np.zeros(nkeys + 1, np.int32)
    np.cumsum(cnt, out=startb[1:])
    rank = np.arange(E, dtype=np.int32)
    rank -= startb[key_s]
    keep = rank < CAP

    # per-(core, stage) blob assembly + async puts; dispatch each stage as
    # its blobs are issued so early outputs materialize while later stages
    # still upload (full-duplex tunnel)
    outs = []
    shard_list = []
    spill = []
    for s in range(NSTAGES):
        parts = []
        for c in range(NCORES):
            cs = c * NSTAGES + s
            lo = startb[cs * NB_S]
            hi = startb[(cs + 1) * NB_S]
            sl = slice(lo, hi)
            kl = keep[sl]
            if kl.all():
                bkt = key_s[sl].astype(np.int32)
                bkt -= cs * NB_S
                pos = bkt * CAP + rank[sl]
                cw, dlv, vav = cw_s[sl], dl_s[sl], va_s[sl]
            else:
                spill.append((sl, kl))
                bkt = key_s[sl].astype(np.int32) - cs * NB_S
                pos = bkt[kl] * CAP + rank[sl][kl]
                cw, dlv, vav = cw_s[sl][kl], dl_s[sl][kl], va_s[sl][kl]
            blob = np.zeros(EB, np.uint8)
            gi = blob[:2 * ETOT_S].view(np.int16)
            gi[(pos % 16) * (ETOT_S // 16) + pos // 16] = cw
            g128 = (pos % 128) * (ETOT_S // 128) + pos // 128
            blob[2 * ETOT_S:3 * ETOT_S].view(np.int8)[g128] = dlv
            blob[3 * ETOT_S:].view(np.int8)[g128] = vav
            parts.append(jax.device_put(
                blob.view(np.int8).reshape(128, EB // 128), _DEV.devices[c]))
        eblob_dev = jax.make_array_from_single_device_arrays(
            (NCORES * 128, EB // 128), _DEV.sh, parts)
        (o,) = _DEV.runner(xq_dev, eblob_dev,
                           _DEV.iota_dev, _DEV.zout_dev)
        outs.append(o)
        for sd in o.addressable_shards:
            sd.data.copy_to_host_async()
            shard_list.append((s, sd.index[0].start or 0, sd.data))

    # ---- fetch + dequantize into the final buffer
    try:
        out = np.empty((n, D), np.float32)
        for s, row0, data in shard_list:
            buf = np.asarray(data)            # [OROWS, OCOLS] int8
            c = row0 // OROWS
            glo = c * SHARD + s * OROWS       # global output row of buf[0]
            ghi = min(glo + OROWS, (c + 1) * SHARD, n)
            if ghi <= glo:
                continue
            nr = ghi - glo
            sc = np.ascontiguousarray(buf[:nr, D:]).view(np.float32)
            np.multiply(buf[:nr, :D], sc, out=out[glo:ghi])
    except Exception:
        return _host_reference(x, W, rows, cols, vals)

    if spill:  # host correction for overflowing buckets (exact)
        h = x @ W.T
        for sl, kl in spill:
            sp = np.flatnonzero(~kl) + sl.start
            osp = order[sp]
            np.add.at(out, rows[osp], vals[osp][:, None] * h[cols[osp]])

    return out


_warmup()


# revision 39
# speedup vs baseline: 1.0420x; 1.0420x over previous
"""GCNConv on 8 Trainium2 NeuronCores (Bass/Tile) — transfer-pipelined.

The device compute (projection + edge aggregation) takes <100ms; the axon
tunnel (~40MB/s each way) dominates, so the kernel is engineered around the
wire:

 - x is quantized host-side to int8 with per-row scales (12.8MB up instead
   of 25.7MB bf16); the dequant scale is fused into the projection's
   PSUM->SBUF copy (activation scale).
 - The output is quantized on-device to int8 with per-row scales packed as
   4 extra bytes per row (13.2MB down instead of 25.7MB), dequantized on
   the host. Measured end-to-end rel err 0.9e-2 vs the 2e-2 gate.
 - No donation: output params are persistent device-resident zero buffers,
   so no 25.7MB zero upload per call.
 - Work is split into four stage invocations of one program (each
   re-projects + AllGathers h, then aggregates a quarter of the
   destination tiles) so early stages' output fetches overlap later
   stages' uploads/exec on the full-duplex tunnel.
 - Edge values are quantized to int8 with one global scale (uniform [0,1)
   values suit linear fixed-point; fp8's log spacing fails the gate). The
   x-row scales, W (bf16), and the vals scale are bit-packed as extra
   int8 columns of the xq upload and read via SBUF AP bitcasts, so each
   core's entire dense payload is one device_put.
 - Per-(core,stage) edge payloads are packed into single int8 blobs
   (gidx int16 region | dest-local int8 region | vals int8 region, read on
   device through SBUF AP bitcasts) and uploaded with async device_puts
   issued core-by-core the moment the host finishes each piece, so the
   wire never idles behind host prep.

Aggregation math is the baseline's exact scheme: edges bucketed by
(dest-core, stage-tile, col-chunk) with fixed CAP=640 per bucket (seed-0
max occupancy is 595; overflow is corrected exactly on the host), gpsimd
dma_gather pulls h[cols] per 640-edge bucket, the vector engine scales by
vals and builds selection matrices, and the tensor engine accumulates
S^T @ M into one PSUM tile per dest tile — an exact f32 segment-sum.
"""
import ctypes
import sys

import numpy as np
import ml_dtypes

try:
    # keep large numpy buffers on the heap arena across calls: without this
    # glibc mmap/munmaps every >128KB allocation, so each kernel() call
    # page-faults ~80MB of fresh temporaries inside the timed region
    _libc = ctypes.CDLL("libc.so.6", use_errno=True)
    _libc.mallopt(-3, 1 << 28)   # M_MMAP_THRESHOLD = 256MB
    _libc.mallopt(-1, 1 << 28)   # M_TRIM_THRESHOLD = 256MB
except Exception:
    pass

sys.path.insert(0, "/opt/trn_rl_repo")

import concourse.bass as bass
import concourse.bacc as bacc
import concourse.mybir as mybir
import concourse.tile as tile

F32 = mybir.dt.float32
BF16 = mybir.dt.bfloat16
I16 = mybir.dt.int16
I8 = mybir.dt.int8

N_NODES = 100000
D = 128
NCORES = 8
SHARD = 12544                  # 98 tiles of 128 rows per core
N_TILES = SHARD // D
N_PAD = SHARD * NCORES         # 100352
CHUNK = 25088                  # col-index window (int16-safe)
N_CHUNKS = N_PAD // CHUNK
CAP = 640                      # edges per (dest tile, col chunk) bucket
NSTAGES = 4
T_STAGE = 25                   # tiles per stage (98 real + 2 pad)
NB_S = T_STAGE * N_CHUNKS      # buckets per (core, stage), tile-major
ETOT_S = NB_S * CAP            # padded edge slots per (core, stage)
EB = 4 * ETOT_S                # blob bytes: gidx 2E | dl 1E | vals 1E
WSC_COLS = N_TILES + 64 + 1    # xsc f32 | W.T bf16 bitcast | vals scale
OCOLS = D + 4                  # int8 data + f32 scale bytes per row
OROWS = T_STAGE * D            # 3200 output rows per (core, stage)
XQW_COLS = SHARD + 4 * WSC_COLS  # xq int8 | wsc f32 bit-packed per core
MAGIC = 12582912.0             # 1.5 * 2**23: fast float32 round-to-int


def _build_nc():
    """Stage program: project h = x @ W.T (dequantizing int8 x), AllGather
    h, aggregate T_STAGE dest tiles from the edge blob (tile-major, one
    640-edge gather per (tile, chunk) bucket, one PSUM tile per dest
    tile), emit int8 output rows with packed per-row f32 scales."""
    nc = bacc.Bacc("TRN2", target_bir_lowering=False, debug=False,
                   num_devices=NCORES)

    xq = nc.dram_tensor("xq", [D, XQW_COLS], I8, kind="ExternalInput")
    eblob = nc.dram_tensor("eblob", [128, EB // 128], I8,
                           kind="ExternalInput")
    iota = nc.dram_tensor("iota", [128, 128], I8, kind="ExternalInput")
    outq = nc.dram_tensor("outq", [OROWS, OCOLS], I8, kind="ExternalOutput")

    h_local = nc.dram_tensor("h_local", [SHARD, D], BF16)
    h_full = nc.dram_tensor("h_full", [N_PAD, D], BF16, addr_space="Shared")

    NBLK = CAP // 128          # 5 gather blocks per bucket

    with tile.TileContext(nc) as tc:
        with (
            tc.tile_pool(name="big", bufs=1) as big_pool,
            tc.tile_pool(name="proj", bufs=4) as proj_pool,
            tc.tile_pool(name="idx", bufs=3) as idx_pool,
            tc.tile_pool(name="gat", bufs=2) as gat_pool,
            tc.tile_pool(name="sca", bufs=2) as sca_pool,
            tc.tile_pool(name="sel", bufs=2) as sel_pool,
            tc.tile_pool(name="psum", bufs=4,
                         space=bass.MemorySpace.PSUM) as psum_pool,
            tc.tile_pool(name="fin", bufs=4) as fin_pool,
            tc.tile_pool(name="qs", bufs=4) as qs_pool,
        ):
            xq_sb = big_pool.tile([D, XQW_COLS], I8)
            dl_sb = big_pool.tile([128, ETOT_S // 128], I8)
            va_sb = big_pool.tile([128, ETOT_S // 128], I8)
            vab = big_pool.tile([128, ETOT_S // 128], BF16)
            iota_sb = big_pool.tile([128, 128], I8)

            nc.sync.dma_start(xq_sb[:], xq[:])
            nc.sync.dma_start(
                dl_sb[:],
                bass.AP(eblob, 2 * ETOT_S,
                        [[ETOT_S // 128, 128], [1, ETOT_S // 128]]),
            )
            nc.sync.dma_start(
                va_sb[:],
                bass.AP(eblob, 3 * ETOT_S,
                        [[ETOT_S // 128, 128], [1, ETOT_S // 128]]),
            )
            nc.sync.dma_start(iota_sb[:], iota[:])
            # bitcast views into the wsc region packed after the xq columns
            wt_bf = xq_sb[:, SHARD + 4 * N_TILES:
                          SHARD + 4 * (N_TILES + 64)].bitcast(BF16)

            def _xsc(t):
                return xq_sb[:, SHARD + 4 * t:SHARD + 4 * (t + 1)].bitcast(F32)

            # dequantize int8 vals with the global scale (wsc last col)
            nc.scalar.activation(
                vab[:], va_sb[:], mybir.ActivationFunctionType.Copy,
                scale=_xsc(WSC_COLS - 1),
            )

            # projection: h_local = (xq @ W.T) * xscale, 128-row tiles
            for t in range(N_TILES):
                xb = proj_pool.tile([D, D], BF16)
                nc.scalar.copy(xb[:], xq_sb[:, t * D:(t + 1) * D])
                ps = psum_pool.tile([D, D], F32)
                nc.tensor.matmul(out=ps[:], lhsT=xb[:], rhs=wt_bf,
                                 start=True, stop=True)
                ht = proj_pool.tile([D, D], BF16)
                nc.scalar.activation(
                    ht[:], ps[:], mybir.ActivationFunctionType.Copy,
                    scale=_xsc(t),
                )
                nc.sync.dma_start(
                    bass.AP(h_local, t * D * D, [[D, D], [1, D]]), ht[:]
                )

            nc.gpsimd.collective_compute(
                "AllGather",
                mybir.AluOpType.bypass,
                replica_groups=[list(range(NCORES))],
                ins=[h_local[:]],
                outs=[h_full[:]],
            )

            # aggregation: tile-major, one 640-edge bucket per (tile, chunk)
            for t in range(T_STAGE):
                ps = psum_pool.tile([D, D], F32)
                for k in range(N_CHUNKS):
                    stream_off = (t * N_CHUNKS + k) * CAP
                    v0 = stream_off // 128
                    gq = idx_pool.tile([128, CAP // 8], I8)
                    nc.sync.dma_start(
                        gq[:],
                        bass.AP(eblob, stream_off // 8,
                                [[0, 8], [ETOT_S // 8, 16], [1, CAP // 8]]),
                    )
                    g = gat_pool.tile([128, NBLK, D], BF16)
                    nc.gpsimd.dma_gather(
                        out_ap=g[:],
                        in_ap=bass.AP(h_full, k * CHUNK * D,
                                      [[D, CHUNK], [1, D]]),
                        idxs_ap=gq[:].bitcast(I16),
                        num_idxs=CAP,
                        num_idxs_reg=CAP,
                        elem_size=D,
                    )
                    gs = sca_pool.tile([128, NBLK, D], BF16)
                    nc.vector.tensor_tensor(
                        out=gs[:],
                        in0=g[:],
                        in1=vab[:, v0:v0 + NBLK]
                        .unsqueeze(2).to_broadcast([128, NBLK, D]),
                        op=mybir.AluOpType.mult,
                    )
                    sel = sel_pool.tile([128, NBLK, D], BF16)
                    nc.vector.tensor_tensor(
                        out=sel[:],
                        in0=dl_sb[:, v0:v0 + NBLK]
                        .unsqueeze(2).to_broadcast([128, NBLK, D]),
                        in1=iota_sb[:].unsqueeze(1)
                        .to_broadcast([128, NBLK, D]),
                        op=mybir.AluOpType.is_equal,
                    )
                    for j in range(NBLK):
                        nc.tensor.matmul(
                            out=ps[:],
                            lhsT=sel[:, j, :],
                            rhs=gs[:, j, :],
                            start=(k == 0 and j == 0),
                            stop=(k == N_CHUNKS - 1 and j == NBLK - 1),
                        )

                # int8 quantization with per-row scale, packed [q | scale]
                m = qs_pool.tile([128, 1], F32)
                nc.vector.tensor_reduce(
                    out=m[:], in_=ps[:],
                    axis=mybir.AxisListType.X, op=mybir.AluOpType.max,
                    apply_absolute_value=True,
                )
                nc.vector.tensor_scalar_max(m[:], m[:], 1e-30)
                inv = qs_pool.tile([128, 1], F32)
                nc.vector.reciprocal(inv[:], m[:])
                inv127 = qs_pool.tile([128, 1], F32)
                nc.vector.tensor_scalar_mul(inv127[:], inv[:], 127.0)
                fin = fin_pool.tile([128, OCOLS], I8)
                nc.vector.tensor_scalar(
                    out=fin[:, 0:D], in0=ps[:],
                    scalar1=inv127[:], scalar2=None,
                    op0=mybir.AluOpType.mult,
                )
                nc.scalar.mul(fin[:, D:OCOLS].bitcast(F32), m[:], 1.0 / 127.0)
                nc.sync.dma_start(
                    bass.AP(outq, t * D * OCOLS, [[OCOLS, D], [1, OCOLS]]),
                    fin[:],
                )

    nc.compile()
    return nc


_NC = _build_nc()


def _make_runner(nc):
    """Jitted no-donation executor for the stage program."""
    import jax
    from jax.sharding import Mesh, PartitionSpec
    from jax.experimental.shard_map import shard_map
    from concourse import bass2jax

    bass2jax.install_neuronx_cc_hook()
    assert nc.dbg_addr is None

    partition_name = (nc.partition_id_tensor.name
                      if nc.partition_id_tensor else None)
    in_names, out_names, out_avals = [], [], []
    for alloc in nc.m.functions[0].allocations:
        if not isinstance(alloc, mybir.MemoryLocationSet):
            continue
        name = alloc.memorylocations[0].name
        if alloc.kind == "ExternalInput":
            if name != partition_name:
                in_names.append(name)
        elif alloc.kind == "ExternalOutput":
            shape = tuple(alloc.tensor_shape)
            dtype = mybir.dt.np(alloc.dtype)
            out_names.append(name)
            out_avals.append(jax.core.ShapedArray(shape, dtype))
    n_params = len(in_names)
    all_in = in_names + out_names
    if partition_name is not None:
        all_in.append(partition_name)

    def _body(*args):
        operands = list(args)
        if partition_name is not None:
            operands.append(bass2jax.partition_id_tensor())
        outs = bass2jax._bass_exec_p.bind(
            *operands,
            out_avals=tuple(out_avals),
            in_names=tuple(all_in),
            out_names=tuple(out_names),
            lowering_input_output_aliases=(),
            sim_require_finite=True,
            sim_require_nnan=True,
            nc=nc,
        )
        return tuple(outs)

    devices = jax.devices()[:NCORES]
    mesh = Mesh(np.asarray(devices), ("core",))
    spec = PartitionSpec("core")
    sharded = jax.jit(
        shard_map(_body, mesh=mesh, in_specs=(spec,) * (n_params + 1),
                  out_specs=(spec,), check_rep=False),
        keep_unused=True,
    )
    return sharded


_IOTA8 = np.ascontiguousarray(
    np.broadcast_to(np.arange(128, dtype=np.int8), (128, 128)))

# per-row-tile key LUT: rows >> 7 -> core*NSTAGES*NB_S + stage*NB_S + tls*4
_R7 = np.arange(N_PAD // D, dtype=np.int32)
_R7_TL = _R7 % N_TILES
_KEYROW = (_R7 // N_TILES * (NSTAGES * NB_S)
           + (_R7_TL // T_STAGE) * NB_S
           + (_R7_TL % T_STAGE) * N_CHUNKS).astype(np.int16)


class _Device:
    """Holds the jit and the persistent device-resident constants."""

    def __init__(self):
        import jax
        from jax.sharding import Mesh, NamedSharding, PartitionSpec

        self.jax = jax
        self.devices = jax.devices()[:NCORES]
        mesh = Mesh(np.asarray(self.devices), ("core",))
        self.sh = NamedSharding(mesh, PartitionSpec("core"))
        self.runner = _make_runner(_NC)
        self.iota_dev = jax.device_put(
            np.concatenate([_IOTA8] * NCORES, axis=0), self.sh)
        self.zout_dev = jax.device_put(
            np.zeros((NCORES * OROWS, OCOLS), np.int8), self.sh)
        jax.block_until_ready(self.iota_dev)
        jax.block_until_ready(self.zout_dev)

    def put_global(self, arr):
        return self.jax.device_put(arr, self.sh)


try:
    _DEV = _Device()
except Exception:
    _DEV = None


def _warmup():
    if _DEV is None:
        return
    try:
        e = np.arange(1600000, dtype=np.int32)
        synth = {
            "x": np.zeros((N_NODES, D), np.float32),
            "W": np.zeros((D, D), np.float32),
            "adj_rows": e % N_NODES,
            "adj_cols": (e * 7) % N_NODES,
            "adj_vals": np.zeros(1600000, np.float32),
        }
        for _ in range(2):
            kernel(**synth)
    except Exception:
        pass


def _host_reference(x, W, rows, cols, vals):
    """Exact full-host fallback (used only if the device path fails)."""
    h = x @ W.T
    order = np.argsort(rows, kind="stable")
    rows_s = rows[order]
    msg = h[cols[order]] * vals[order][:, None]
    boundaries = np.searchsorted(rows_s, np.arange(N_NODES)).astype(np.int64)
    np.clip(boundaries, 0, max(len(rows_s) - 1, 0), out=boundaries)
    out = np.add.reduceat(msg, boundaries, axis=0)
    counts = np.bincount(rows, minlength=N_NODES)
    out[counts == 0] = 0.0
    return out.astype(np.float32)


def kernel(x, W, adj_rows, adj_cols, adj_vals):
    x = np.asarray(x, dtype=np.float32)
    W = np.asarray(W, dtype=np.float32)
    rows = np.asarray(adj_rows).astype(np.int32, copy=False)
    cols = np.asarray(adj_cols).astype(np.int32, copy=False)
    vals = np.asarray(adj_vals, dtype=np.float32)
    n = x.shape[0]

    if _DEV is None:
        return _host_reference(x, W, rows, cols, vals)

    jax = _DEV.jax

    # ---- x: per-row int8 quantization + packed scales/W, per-core
    # async upload pipeline (wsc region bit-packed after the xq columns)
    vsc = max(float(np.abs(vals).max()), 1e-30) / 127.0
    wtb = np.ascontiguousarray(W.T).astype(ml_dtypes.bfloat16)
    wtf = wtb.view(np.float32)               # [128, 64] bit-packed bf16
    wsc_c = np.empty((128, WSC_COLS), np.float32)
    wsc_c[:, N_TILES:N_TILES + 64] = wtf
    wsc_c[:, WSC_COLS - 1] = vsc
    xq_parts = []
    for c in range(NCORES):
        lo = c * SHARD
        hi = min(lo + SHARD, n)
        xc = x[lo:hi]
        m = np.abs(xc).max(axis=1)
        np.maximum(m, 1e-30, out=m)
        s = m * (1.0 / 127.0)
        buf = xc * (1.0 / s)[:, None]
        buf += MAGIC
        i32 = buf.view(np.int32)
        i32 -= 0x4B400000
        q8 = i32.astype(np.int8)
        qt = np.zeros((D, XQW_COLS), np.int8)
        qt[:, :xc.shape[0]] = q8.T
        s_pad = np.full(SHARD, 1.0, np.float32)
        s_pad[:xc.shape[0]] = s
        wsc_c[:, :N_TILES] = s_pad.reshape(N_TILES, 128).T
        qt[:, SHARD:] = wsc_c.view(np.int8)
        xq_parts.append(jax.device_put(qt, _DEV.devices[c]))
    xq_dev = jax.make_array_from_single_device_arrays(
        (NCORES * D, XQW_COLS), _DEV.sh, xq_parts)

    # ---- edge bucketing: (core, stage, tile, chunk) with CAP slots each
    E = rows.shape[0]
    key = _KEYROW[rows >> 7] + (cols // CHUNK).astype(np.int16)
    order = np.argsort(key, kind="stable")
    key_s = key[order]
    dl_s = (rows & 127).astype(np.int8)[order]
    cw_s = (cols % CHUNK).astype(np.int16)[order]
    vq = vals * (1.0 / vsc)
    vq += MAGIC
    vqi = vq.view(np.int32)
    vqi -= 0x4B400000
    va_s = vqi.astype(np.int8)[order]

    nkeys = NCORES * NSTAGES * NB_S
    cnt = np.bincount(key_s, minlength=nkeys)
    startb = np.zeros(nkeys + 1, np.int32)
    np.cumsum(cnt, out=startb[1:])
    rank = np.arange(E, dtype=np.int32)
    rank -= startb[key_s]
    keep = rank < CAP

    # per-(core, stage) blob assembly + async puts; dispatch each stage as
    # its blobs are issued so early outputs materialize while later stages
    # still upload (full-duplex tunnel)
    outs = []
    shard_list = []
    spill = []
    for s in range(NSTAGES):
        parts = []
        for c in range(NCORES):
            cs = c * NSTAGES + s
            lo = startb[cs * NB_S]
            hi = startb[(cs + 1) * NB_S]
            sl = slice(lo, hi)
            kl = keep[sl]
            if kl.all():
                bkt = key_s[sl].astype(np.int32)
                bkt -= cs * NB_S
                pos = bkt * CAP + rank[sl]
                cw, dlv, vav = cw_s[sl], dl_s[sl], va_s[sl]
            else:
                spill.append((sl, kl))
                bkt = key_s[sl].astype(np.int32) - cs * NB_S
                pos = bkt[kl] * CAP + rank[sl][kl]
                cw, dlv, vav = cw_s[sl][kl], dl_s[sl][kl], va_s[sl][kl]
            blob = np.zeros(EB, np.uint8)
            gi = blob[:2 * ETOT_S].view(np.int16)
            gi[(pos % 16) * (ETOT_S // 16) + pos // 16] = cw
            g128 = (pos % 128) * (ETOT_S // 128) + pos // 128
            blob[2 * ETOT_S:3 * ETOT_S].view(np.int8)[g128] = dlv
            blob[3 * ETOT_S:].view(np.int8)[g128] = vav
            parts.append(jax.device_put(
                blob.view(np.int8).reshape(128, EB // 128), _DEV.devices[c]))
        eblob_dev = jax.make_array_from_single_device_arrays(
            (NCORES * 128, EB // 128), _DEV.sh, parts)
        (o,) = _DEV.runner(xq_dev, eblob_dev,
                           _DEV.iota_dev, _DEV.zout_dev)
        outs.append(o)
        for sd in o.addressable_shards:
            sd.data.copy_to_host_async()
            shard_list.append((s, sd.index[0].start or 0, sd.data))

    # ---- fetch + dequantize into the final buffer
    try:
        out = np.empty((n, D), np.float32)
        for s, row0, data in shard_list:
            buf = np.asarray(data)            # [OROWS, OCOLS] int8
            c = row0 // OROWS
            glo = c * SHARD + s * OROWS       # global output row of buf[0]
            ghi = min(glo + OROWS, (c + 1) * SHARD, n)
            if ghi <= glo:
                continue
            nr = ghi - glo
            sc = np.ascontiguousarray(buf[:nr, D:]).view(np.float32)
            np.multiply(buf[:nr, :D], sc, out=out[glo:ghi])
    except Exception:
        return _host_reference(x, W, rows, cols, vals)

    if spill:  # host correction for overflowing buckets (exact)
        h = x @ W.T
        for sl, kl in spill:
            sp = np.flatnonzero(~kl) + sl.start
            osp = order[sp]
            np.add.at(out, rows[osp], vals[osp][:, None] * h[cols[osp]])

    return out


_warmup()
